# revision 1
# baseline (speedup 1.0000x reference)
"""Trainium2 Bass kernel for nn_BiologicalMemory (retrieval_knn).

Full-input contract: kernel(**inputs) takes the complete unsharded inputs and
returns the complete [4096] output. Internally shards across 8 NeuronCores:
  - memory_bank / importance / age row-sharded (1024 rows per core)
  - W_dec.T column-sharded (each core produces 512 output elements)
  - query replicated (pre-broadcast to 128 partitions on host)

Phase A (scores) runs on DVE (affine_mul_reduce: fused dot+reduce) and ACT
(square+accumulate) from the natural row-major layout, so it is DMA-bound.
One AllGather exchanges each core's local top-8 candidate rows -- shipped as
a bf16 hi+lo pair (same bytes as fp32, reconstructed exactly by two
accumulating bf16 PE passes) -- plus the fp32 scores bit-packed. Every core
reduces the 64 candidates to the global top-8 by score-thresholding, means
the winning rows, and decodes its own 512-wide output slice with fp32r
matmuls (full-rate PE fp32; operands must be DMA'd straight from DRAM, hence
the wt dtype and the DRAM bounce of the retrieved vector).
"""

import numpy as np

import concourse.bass as bass
import concourse.mybir as mybir
import concourse.tile as tile
from concourse import bacc
from concourse.bass import ts
from concourse.bass_utils import run_bass_kernel_spmd
from concourse.masks import make_identity

DIM = 4096
CAP = 8192
NCORES = 8
RPC = CAP // NCORES   # rows per core        (1024)
OPC = DIM // NCORES   # output elems / core  (512)
K = 8                 # top_k
NT = RPC // 128       # row tiles per core   (8)
DC = DIM // 128       # d-chunks             (32)
HD = DIM // 2         # DMA split point      (2048)
CCB = 2 * K * DIM + 2 * K  # collective payload, bf16 elems (hi+lo rows+vals)
EPS = 1e-8

F32 = mybir.dt.float32
F32R = mybir.dt.float32r
BF16 = mybir.dt.bfloat16
U32 = mybir.dt.uint32
AF = mybir.ActivationFunctionType
ALU = mybir.AluOpType


def _build_nc():
    nc = bacc.Bacc(None, num_devices=NCORES, debug=False)
    _emit(nc)
    nc.compile()
    return nc


def _emit(nc):
    mrow = nc.dram_tensor("mrow", [RPC, DIM], F32, kind="ExternalInput")
    qb_d = nc.dram_tensor("qb", [128, DIM], F32, kind="ExternalInput")
    impa = nc.dram_tensor("impa", [128, NT], F32, kind="ExternalInput")
    agev = nc.dram_tensor("agev", [128, NT], F32, kind="ExternalInput")
    wt = nc.dram_tensor("wt", [DIM, OPC], F32R, kind="ExternalInput")
    bcv = nc.dram_tensor("bcv", [1, OPC], F32, kind="ExternalInput")
    out = nc.dram_tensor("out", [1, OPC], F32, kind="ExternalOutput")

    with tile.TileContext(nc) as tc:
        with (
            tc.tile_pool(name="persist", bufs=1) as pp,
            tc.tile_pool(name="mtp", bufs=2) as mtp,
            tc.tile_pool(name="scr", bufs=1) as scrp,
            tc.tile_pool(name="small", bufs=1) as sp,
            tc.tile_pool(name="psum", bufs=1, space="PSUM") as psp,
            tc.tile_pool(name="dram", bufs=1, space="DRAM") as dp,
        ):
            # ---- phase A streaming: dots (DVE) + sum-of-squares (ACT).
            # Tile DMAs are split across the sync and scalar HWDGE queues.
            dots8 = sp.tile([128, NT], F32, name="dots8")
            ss8 = sp.tile([128, NT], F32, name="ss8")
            qb = pp.tile([128, DIM], F32, name="qb")
            nc.gpsimd.dma_start(qb, qb_d[:, :])
            m_tiles = []
            last_mt_dmas = []
            for t in range(NT):
                m_t = mtp.tile([128, DIM], F32, name="m_t", tag="m")
                d0 = nc.sync.dma_start(m_t[:, :HD], mrow[ts(t, 128), :HD])
                d1 = nc.scalar.dma_start(m_t[:, HD:], mrow[ts(t, 128), HD:])
                if t == NT - 1:
                    last_mt_dmas = [d0, d1]
                m_tiles.append(m_t)

            ident = pp.tile([128, 128], F32, name="ident")
            make_identity(nc, ident)
            imp_sb = sp.tile([128, NT], F32, name="imp_sb")
            nc.gpsimd.dma_start(imp_sb, impa[:, :])
            age_sb = sp.tile([128, NT], F32, name="age_sb")
            nc.gpsimd.dma_start(age_sb, agev[:, :])

            # ||q||^2 on every partition (each qb row is the full q)
            qscr = scrp.tile([128, DIM], BF16, name="qscr", tag="actscr")
            qn2col = sp.tile([128, 1], F32, name="qn2col")
            nc.scalar.activation(qscr, qb, AF.Square, accum_out=qn2col)

            ie8 = sp.tile([128, NT], F32, name="ie8")
            nc.scalar.activation(ie8, age_sb, AF.Exp, scale=-0.001)
            nc.vector.tensor_mul(ie8, ie8, imp_sb)

            for t in range(NT):
                m_t = m_tiles[t]
                dscr = scrp.tile([128, DIM], BF16, name="dscr", tag="dvescr")
                nc.vector.affine_mul_reduce(
                    out=dscr,
                    accum_out=dots8[:, t : t + 1],
                    in0=m_t,
                    in1=qb,
                    scale=1.0,
                    bias=0.0,
                )
                ascr = scrp.tile([128, DIM], BF16, name="ascr", tag="actscr")
                nc.scalar.activation(
                    ascr, m_t, AF.Square, accum_out=ss8[:, t : t + 1]
                )

            # ---- decoder slice prefetch (own queue; overlaps the collective)
            # hold the decoder prefetch until phase-A streaming is done so it
            # doesn't steal HBM bandwidth from the score pass
            from concourse.tile import add_dep_helper
            wt_sb = pp.tile([128, DC, OPC], F32R, name="wt_sb")
            for c in range(DC):
                wdma = nc.gpsimd.dma_start(wt_sb[:, c, :], wt[ts(c, 128), :])
                if c == 0:
                    for d in last_mt_dmas:
                        add_dep_helper(wdma.ins, d.ins, sync=True,
                                       reason="wt prefetch after phase-A traffic")

            # ---- scores [128, 8]: s = dots / max(sqrt(ssq*qn2), eps) * ie
            den = sp.tile([128, NT], F32, name="den")
            nc.vector.tensor_scalar_mul(den, ss8, qn2col)
            nc.scalar.sqrt(den, den)
            nc.vector.tensor_scalar_max(den, den, EPS)
            rden = sp.tile([128, NT], F32, name="rden")
            nc.vector.reciprocal(rden, den)
            s8 = sp.tile([128, NT], F32, name="s8")
            nc.vector.tensor_mul(s8, dots8, rden)
            nc.vector.tensor_mul(s8, s8, ie8)

            # ---- flatten scores to [1, 1024] in row order (r = t*128 + p)
            st_ps = psp.tile([NT, 128], F32, name="st_ps", tag="pT")
            nc.tensor.transpose(st_ps, s8, ident)
            st = sp.tile([NT, 128], F32, name="st")
            nc.vector.tensor_copy(st, st_ps)
            sflat = sp.tile([1, RPC], F32, name="sflat")
            nc.sync.dma_start(sflat, st)

            # ---- local top-8
            mx8 = sp.tile([1, 8], F32, name="mx8")
            nc.vector.max(out=mx8, in_=sflat)
            idx8 = sp.tile([1, 8], U32, name="idx8")
            nc.vector.max_index(out=idx8, in_max=mx8, in_values=sflat)
            idxc = sp.tile([8, 1], U32, name="idxc")
            nc.sync.dma_start(idxc, idx8)

            # ---- gather local top-8 rows; split into bf16 hi + lo halves
            rows8 = sp.tile([8, DIM], F32, name="rows8")
            nc.gpsimd.indirect_dma_start(
                out=rows8[:],
                out_offset=None,
                in_=mrow[:, :],
                in_offset=bass.IndirectOffsetOnAxis(ap=idxc[:, :1], axis=0),
            )
            rows8_hi = sp.tile([8, DIM], BF16, name="rows8_hi")
            nc.vector.tensor_copy(rows8_hi, rows8)
            rows8_lo = sp.tile([8, DIM], BF16, name="rows8_lo")
            nc.vector.tensor_sub(rows8_lo, rows8, rows8_hi)

            # ---- AllGather candidates: [hi rows | lo rows | fp32 vals packed]
            cc_in = dp.tile([CCB], BF16, name="cc_in")
            cc_out = dp.tile([NCORES * CCB], BF16, name="cc_out", addr_space="Shared")
            nc.sync.dma_start(
                cc_in[: K * DIM].rearrange("(r d) -> r d", d=DIM), rows8_hi
            )
            nc.scalar.dma_start(
                cc_in[K * DIM : 2 * K * DIM].rearrange("(r d) -> r d", d=DIM),
                rows8_lo,
            )
            nc.sync.dma_start(
                cc_in[2 * K * DIM :].unsqueeze(0), mx8.bitcast(BF16)
            )
            nc.gpsimd.collective_compute(
                "AllGather",
                ALU.bypass,
                replica_groups=[list(range(NCORES))],
                ins=[cc_in.opt()],
                outs=[cc_out.opt()],
            )
            cc8 = cc_out.rearrange("(c x) -> c x", x=CCB)

            # ---- global top-8 among the 64 candidates, by score threshold
            vals64 = sp.tile([1, 64], F32, name="vals64")
            nc.sync.dma_start(vals64, cc8[:, 2 * K * DIM :].bitcast(F32))
            vals64c = sp.tile([64, 1], F32, name="vals64c")
            nc.scalar.dma_start(vals64c, cc8[:, 2 * K * DIM :].bitcast(F32))
            gv8 = sp.tile([1, 8], F32, name="gv8")
            nc.vector.max(out=gv8, in_=vals64)
            thr = sp.tile([64, 1], F32, name="thr")
            nc.gpsimd.partition_broadcast(thr, gv8[0:1, 7:8])
            w64b = sp.tile([64, 1], BF16, name="w64b")
            nc.vector.tensor_scalar(
                w64b, vals64c, thr, 1.0 / K, op0=ALU.is_ge, op1=ALU.mult
            )

            # ---- candidate rows (hi/lo) spread across the three DMA queues
            rows64h = pp.tile([64, DIM], BF16, name="rows64h")
            rows64l = pp.tile([64, DIM], BF16, name="rows64l")
            qs = [nc.sync, nc.scalar, nc.gpsimd]
            for c in range(NCORES):
                qs[c % 3].dma_start(
                    rows64h[ts(c, K), :],
                    cc8[c, : K * DIM].rearrange("(r d) -> r d", d=DIM),
                )
                qs[(c + 1) % 3].dma_start(
                    rows64l[ts(c, K), :],
                    cc8[c, K * DIM : 2 * K * DIM].rearrange("(r d) -> r d", d=DIM),
                )

            # ---- retrieved = w64 . (hi + lo), accumulated in PSUM fp32,
            #      produced directly in [128, 32] layout
            ret_ps = psp.tile([128, DC], F32, name="ret_ps", tag="pA")
            for c in range(DC):
                nc.tensor.matmul(
                    ret_ps[:, c : c + 1],
                    lhsT=rows64h[:, ts(c, 128)],
                    rhs=w64b,
                    start=True,
                    stop=False,
                )
                nc.tensor.matmul(
                    ret_ps[:, c : c + 1],
                    lhsT=rows64l[:, ts(c, 128)],
                    rhs=w64b,
                    start=False,
                    stop=True,
                )
            ret = sp.tile([128, DC], F32, name="ret")
            nc.vector.tensor_copy(ret, ret_ps)
            # bounce through DRAM so the decode lhsT is a legal f32r operand
            rscr = dp.tile([128, DC], F32R, name="rscr")
            nc.sync.dma_start(rscr, ret.bitcast(F32R))
            ret_r = sp.tile([128, DC], F32R, name="ret_r")
            nc.sync.dma_start(ret_r, rscr[:, :])

            # ---- decode: out_slice = retrieved @ W_dec[slice].T + b[slice]
            out_ps = psp.tile([1, OPC], F32, name="out_ps", tag="pout")
            for c in range(DC):
                nc.tensor.matmul(
                    out_ps,
                    lhsT=ret_r[:, c : c + 1],
                    rhs=wt_sb[:, c, :],
                    start=(c == 0),
                    stop=(c == DC - 1),
                )
            bc_sb = sp.tile([1, OPC], F32, name="bc_sb")
            nc.gpsimd.dma_start(bc_sb, bcv[:, :])
            out_sb = sp.tile([1, OPC], F32, name="out_sb")
            nc.vector.tensor_add(out_sb, out_ps, bc_sb)
            nc.sync.dma_start(out[:, :], out_sb)


_NC_CACHE = {}


def _get_nc():
    if "nc" not in _NC_CACHE:
        _NC_CACHE["nc"] = _build_nc()
    return _NC_CACHE["nc"]


def _make_in_maps(query, memory_bank, importance, age, W_dec, b_dec):
    query = np.ascontiguousarray(np.asarray(query, dtype=np.float32))
    memory_bank = np.ascontiguousarray(np.asarray(memory_bank, dtype=np.float32))
    importance = np.ascontiguousarray(np.asarray(importance, dtype=np.float32))
    age = np.ascontiguousarray(np.asarray(age, dtype=np.float32))
    W_dec = np.ascontiguousarray(np.asarray(W_dec, dtype=np.float32))
    b_dec = np.ascontiguousarray(np.asarray(b_dec, dtype=np.float32))

    qb = np.ascontiguousarray(np.broadcast_to(query[None, :], (128, DIM)))
    in_maps = []
    for c in range(NCORES):
        rs = slice(c * RPC, (c + 1) * RPC)
        os = slice(c * OPC, (c + 1) * OPC)
        in_maps.append(
            {
                "mrow": np.ascontiguousarray(memory_bank[rs]),
                "qb": qb,
                "impa": np.ascontiguousarray(importance[rs].reshape(NT, 128).T),
                "agev": np.ascontiguousarray(age[rs].reshape(NT, 128).T),
                "wt": np.ascontiguousarray(W_dec[os, :].T),
                "bcv": np.ascontiguousarray(b_dec[os].reshape(1, OPC)),
            }
        )
    return in_maps


def run(inputs, trace=False, **run_kwargs):
    """Build (cached), run on 8 cores, gather. Returns (output, BassKernelResults)."""
    assert int(inputs.get("top_k", K)) == K
    nc = _get_nc()
    in_maps = _make_in_maps(
        inputs["query"],
        inputs["memory_bank"],
        inputs["importance"],
        inputs["age"],
        inputs["W_dec"],
        inputs["b_dec"],
    )
    res = run_bass_kernel_spmd(
        nc, in_maps, core_ids=list(range(NCORES)), trace=trace, **run_kwargs
    )
    out = np.concatenate(
        [res.results[c]["out"].reshape(OPC) for c in range(NCORES)]
    ).astype(np.float32)
    return out, res


def kernel(**inputs) -> np.ndarray:
    out, _ = run(inputs, trace=False)
    return out

